# revision 65
# baseline (speedup 1.0000x reference)
"""Multi-head self-attention Trainium2 kernel (Bass/Tile), batch-parallel
over 8 NeuronCores: one batch element per core, full weights everywhere,
no collectives.

Problem (hardcoded): B=8, L=1024, D=1024, H=16, hd=64, f32 in/out.
  qkv = x @ w_qkv + b_qkv ; per-head scores = q k^T / 8 ; mask ; softmax ;
  out = (P v) heads-merged @ w_out + b_out.

Design notes (v4 — all-bf16, ScalarE-bound softmax loop):
  - ScalarE exp over the 16 x 1024 x 1024 score matrix is the hard floor
    (~(N+352)/1.2 ns per ACT ~= 147us total). The head-pair loop streams
    exp ACTs back-to-back; all matmul work hides under it on TensorE.
    (fp8/DoubleRow was tried: ~2x PE-cheaper but any fp8 stage alone
    costs 2-5e-2 relative error vs the 2e-2 gate — measured proj8=4.8e-2,
    e8=2.4e-2, ot8=3e-2. So everything stays bf16.)
  - Scores: per head K=64; the head pair occupies partition ranges
    0:64 / 64:128 of qt/kt, so the pair's score matmuls row-tile and run
    concurrently in the PE array.
  - PV col-tiles the pair (head A -> PSUM partitions 0:64 via tile col 0,
    head B -> 64:128 via col 64) in two 512-query passes through a single
    1-bank [128, 512] ring; 2x PE concurrency, no ones-column.
  - Softmax denominators come from ones-stationary M=1 matmuls, 4-way
    col-tiled (positions 0/32/64/96 hold the (head, nh) groups), one
    PSUM bank, accumulated chunk-by-chunk alongside the scores. The
    finish chain (PSUM->SBUF copy, row hops to partition 0, fast
    reciprocal, partition_broadcast bands) runs on DVE/Sync/GpSimd
    entirely off the ACT path. partition_broadcast only reads/writes
    from physical partition 0, hence the row-hop DMAs.
  - PV + normalize for pair j are deferred into pair j+1's chunk slots
    (e tiles held one extra pair) and emitted in 4 slices so a pending
    PV at the PE queue head never delays the score matmuls feeding the
    next exp ACT.
  - QKV m-tiles for pair j+1 are computed during pair j as 512-token
    halves through the same po ring (1 bank), freeing 2 banks vs a
    dedicated pool; that buys st bufs=3, whose 3-deep score ladder
    absorbs PE bursts up to ~2.3us without stalling ScalarE.
  - PSUM budget (8 banks): scores st 3x2 + po ring 1 + den 1 = 8.
    Pair 0 bootstraps v' (16 chunk-halves) + q/k tiles through a 2-bank
    ring woven 3-per-slot between its own score chunks; its denominator
    matmuls are deferred to the pair-1 boundary where the den bank opens.
  - All evacuations run on DVE, keeping ScalarE exp-only. wo tiles reuse
    the wv pool rings (wv is dead after the v' chunks). The tail
    interleaves pair-7's drain with out-proj partials (k=0..6) so the
    reciprocal-chain wait never idles the PE.
"""

import sys

import numpy as np

try:
    import concourse.bass as bass  # noqa: F401
except Exception:  # pragma: no cover - defensive path setup
    for p in ("/opt/trn_rl_repo", "/opt/pypackages"):
        if p not in sys.path:
            sys.path.insert(0, p)
    import concourse.bass as bass  # noqa: F401

from contextlib import ExitStack

import concourse.tile as tile
from concourse import bacc, bass_isa, mybir
from concourse.bass_utils import run_bass_kernel_spmd

F32 = mybir.dt.float32
BF16 = mybir.dt.bfloat16

B, L, D = 8, 1024, 1024
H, HD = 16, 64
N_CORES = 8
PART = 128
NK = D // PART  # 8 x-dim contraction chunks
NM = 2 * D // PART  # 16 qk output tiles (q: 0-7, k: 8-15)
NLQ = L // PART  # 8 query tiles
NLK = L // PART  # 8 key tiles
NPAIR = H // 2  # 8 head pairs
QSPLIT = 2  # mtile emitted as 2*QSPLIT token-slices
EXP = mybir.ActivationFunctionType.Exp
RADD = bass_isa.ReduceOp.add


def build_nc(debug=False):
    nc = bacc.Bacc("TRN2", target_bir_lowering=False, debug=False)

    xT = nc.dram_tensor("xT", (D, L), BF16, kind="ExternalInput").ap()
    # wqk_blk[m, p, k, c] = w_qkv[k*128 + p, m*128 + c], m < 16 (q then k)
    wqk_blk = nc.dram_tensor(
        "wqk_blk", (NM, PART, NK, PART), BF16, kind="ExternalInput"
    ).ap()
    # bqk[p, m] = b_qkv[m*128 + p]
    bqk = nc.dram_tensor("bqk", (PART, NM), F32, kind="ExternalInput").ap()
    # wv_blk[k, p, n] = w_v[k*128 + p, n]
    wv_blk = nc.dram_tensor("wv_blk", (NK, PART, D), BF16, kind="ExternalInput").ap()
    # vbias[p, n] = b_v[n] (broadcast over p)
    vbias = nc.dram_tensor("vbias", (PART, D), F32, kind="ExternalInput").ap()
    # maskb[p, c] = 0 / -10000 for key c*128+p
    maskb = nc.dram_tensor("maskb", (PART, NLK), F32, kind="ExternalInput").ap()
    # wo_blk[k, p, n] = w_out[k*128 + p, n]
    wo_blk = nc.dram_tensor("wo_blk", (NK, PART, D), BF16, kind="ExternalInput").ap()
    bout = nc.dram_tensor("bout", (PART, D), F32, kind="ExternalInput").ap()
    Y = nc.dram_tensor("Y", (L, D), F32, kind="ExternalOutput").ap()
    dbg = {}
    if debug:
        for nm, shp, dt in [
            ("dbg_q", (PART, L), BF16), ("dbg_k", (PART, L), BF16),
            ("dbg_vv", (PART, D), BF16), ("dbg_e", (PART, 2 * L), BF16),
            ("dbg_esum", (PART, 2 * L), F32), ("dbg_red", (PART, 2 * L), F32),
            ("dbg_rcp", (PART, 2 * L), F32), ("dbg_ot", (PART, L), BF16),
        ]:
            dbg[nm] = nc.dram_tensor(nm, shp, dt, kind="ExternalOutput").ap()

    with tile.TileContext(nc) as tc, ExitStack() as ctx:
        singles = ctx.enter_context(tc.tile_pool(name="singles", bufs=1))
        mb_sb = singles.tile([PART, NLK], F32)
        nc.sync.dma_start(mb_sb[:], maskb[:, :])
        bqk_sb = singles.tile([PART, NM], F32)
        nc.sync.dma_start(bqk_sb[:], bqk[:, :])
        vbias_sb = singles.tile([PART, D], F32)
        bout_sb = singles.tile([PART, D], F32)

        # ---- persistent tiles ----
        xt_pool = ctx.enter_context(tc.tile_pool(name="xt", bufs=1))
        xt = [xt_pool.tile([PART, L], BF16, tag=f"xt{k}", name=f"xt{k}") for k in range(NK)]
        wv_pool = ctx.enter_context(tc.tile_pool(name="wv", bufs=1))
        wv_sb = [wv_pool.tile([PART, D], BF16, tag=f"wv{k}", name=f"wv{k}") for k in range(NK)]
        qk_pool = ctx.enter_context(tc.tile_pool(name="qk", bufs=1))
        qt = [qk_pool.tile([PART, L], BF16, tag=f"q{j}", name=f"q{j}") for j in range(NPAIR)]
        kt = [qk_pool.tile([PART, L], BF16, tag=f"k{j}", name=f"k{j}") for j in range(NPAIR)]
        vv_pool = ctx.enter_context(tc.tile_pool(name="vv", bufs=1))
        vv = [vv_pool.tile([PART, D], BF16, tag=f"vv{c}", name=f"vv{c}") for c in range(NLK)]
        ot_pool = ctx.enter_context(tc.tile_pool(name="ot", bufs=1))
        ot = [ot_pool.tile([PART, L], BF16, tag=f"ot{j}", name=f"ot{j}") for j in range(NPAIR)]

        # qk weight stream
        wqk_pool = ctx.enter_context(tc.tile_pool(name="wqkp", bufs=4))
        mt_seq = [j + NPAIR * s for j in range(NPAIR) for s in range(2)]  # q0,k0,q1,k1,...
        mt_loaded = {}

        def load_mtile(i):
            if i >= NM:
                return
            m = mt_seq[i]
            wt = wqk_pool.tile([PART, NK * PART], BF16, tag="wqk", name="wt")
            nc.sync.dma_start(wt[:], wqk_blk[m].rearrange("p k c -> p (k c)"))
            mt_loaded[i] = wt

        def compute_mtile(i, psum_pool, tag="mt", part=None):
            """part=None: whole [128,L] tile (2 banks).
            part=0/1: token-halves through a 1-bank [128,512] ring."""
            if i >= NM:
                return None
            m = mt_seq[i]
            dst = qt[m] if m < NPAIR else kt[m - NPAIR]
            if part is None:
                wt = mt_loaded.pop(i)
                ph = psum_pool.tile([PART, L], F32, tag=tag, name="ph")
                for k in range(NK):
                    for nh in range(2):
                        nc.tensor.matmul(
                            ph[:, nh * 512 : (nh + 1) * 512],
                            wt[:, k * PART : (k + 1) * PART],
                            xt[k][:, nh * 512 : (nh + 1) * 512],
                            start=(k == 0),
                            stop=(k == NK - 1),
                        )
                nc.vector.tensor_scalar_add(dst[:], ph[:], bqk_sb[:, m : m + 1])
                return
            wt = mt_loaded[i]
            qw = 512 // QSPLIT
            nsl = slice(part * qw, (part + 1) * qw)
            ph = psum_pool.tile([PART, 512], F32, tag=tag, name="ph")
            for k in range(NK):
                nc.tensor.matmul(
                    ph[:, 0:qw],
                    wt[:, k * PART : (k + 1) * PART],
                    xt[k][:, nsl],
                    start=(k == 0),
                    stop=(k == NK - 1),
                )
            nc.vector.tensor_scalar_add(dst[:, nsl], ph[:, 0:qw], bqk_sb[:, m : m + 1])
            if part == 2 * QSPLIT - 1:
                del mt_loaded[i]

        def compute_vchunk(c, psum_pool, tag="vb"):
            pv = psum_pool.tile([PART, D], F32, tag=tag, name="pv")
            for k in range(NK):
                for g in range(2):
                    nc.tensor.matmul(
                        pv[:, g * 512 : (g + 1) * 512],
                        xt[k][:, c * PART : (c + 1) * PART],
                        wv_sb[k][:, g * 512 : (g + 1) * 512],
                        start=(k == 0),
                        stop=(k == NK - 1),
                    )
            nc.vector.tensor_add(vv[c][:], pv[:], vbias_sb[:])

        def compute_vhalf(c, g, psum_pool):
            """Half of v' chunk c (512 of 1024 v-dims) -> vv[c]."""
            gsl = slice(g * 512, (g + 1) * 512)
            pv = psum_pool.tile([PART, 512], F32, tag="vb", name="pv")
            for k in range(NK):
                nc.tensor.matmul(
                    pv[:],
                    xt[k][:, c * PART : (c + 1) * PART],
                    wv_sb[k][:, gsl],
                    start=(k == 0),
                    stop=(k == NK - 1),
                )
            nc.vector.tensor_add(vv[c][:, gsl], pv[:], vbias_sb[:, gsl])

        # ============ input DMAs (sync queue, consumption order) ============
        load_mtile(0)
        load_mtile(1)
        for k in range(NK):
            nc.sync.dma_start(xt[k][:], xT[k * PART : (k + 1) * PART, :])
        load_mtile(2)
        load_mtile(3)
        for k in range(NK):
            nc.sync.dma_start(wv_sb[k][:], wv_blk[k])
        nc.sync.dma_start(vbias_sb[:], vbias[:, :])

        # ============ SBUF pools for the loop ============
        st_ctx = ExitStack()
        st_pool = st_ctx.enter_context(
            tc.tile_pool(name="stp", bufs=3, space="PSUM", side="right")
        )
        e_pool = ctx.enter_context(tc.tile_pool(name="epool", bufs=11))
        fin_pool = ctx.enter_context(tc.tile_pool(name="fin", bufs=1))
        ones_sb = singles.tile([PART, 1], BF16)
        nc.vector.memset(ones_sb[:], 1.0)

        def emit_scores(j, c, e_t):
            """Score matmuls + exp ACTs for chunk c of head pair j."""
            csl = slice(c * PART, (c + 1) * PART)
            for hh in range(2):  # head A (rows 0:64) / head B (64:128)
                ro = hh * HD
                st = st_pool.tile([PART, L], F32, tag="st", name="st")
                for nh in range(2):
                    nsl = slice(nh * 512, (nh + 1) * 512)
                    nc.tensor.matmul(
                        st[:, nsl],
                        kt[j][ro : ro + HD, csl],
                        qt[j][ro : ro + HD, nsl],
                        start=True,
                        stop=True,
                    )
                nc.scalar.activation(
                    e_t[:, hh * L : (hh + 1) * L],
                    st[:],
                    EXP,
                    bias=mb_sb[:, c : c + 1],
                    scale=1.0 / 8.0,
                )

        def emit_den(c, e_t, den_t):
            """Accumulate softmax denominators for chunk c into the den PSUM
            tile: ones-stationary M=1 matmuls, 4-way col-tiled so the four
            (head, nh) groups run concurrently at array columns 0/32/64/96."""
            for hh in range(2):
                for nh in range(2):
                    pos = 32 * (2 * hh + nh)
                    nc.tensor.matmul(
                        den_t[pos : pos + 1, :],
                        ones_sb[:],
                        e_t[:, hh * L + nh * 512 : hh * L + (nh + 1) * 512],
                        start=(c == 0),
                        stop=(c == NLK - 1),
                        tile_position=(0, pos),
                    )

        def emit_finish_prep(j, den_t):
            """den PSUM rows 0/32/64/96 -> partition-0 tiles -> recip ->
            broadcast bands. Frees the den bank via small DMAs."""
            dsb = fin_pool.tile([PART, 512], F32, tag="dsb", name="dsb")
            nc.vector.tensor_copy(dsb[:], den_t[:])
            dr, rc = [dsb[0:1, :]], []
            for i in range(1, 4):
                t = fin_pool.tile([1, 512], F32, tag=f"dr{i}", name=f"dr{i}")
                nc.sync.dma_start(t[:], dsb[32 * i : 32 * i + 1, :])
                dr.append(t)
            with nc.allow_low_precision(reason="softmax denom reciprocal"):
                for i in range(4):
                    t = fin_pool.tile([1, 512], F32, tag=f"rc{i}", name=f"rc{i}")
                    nc.vector.reciprocal_approx_fast(t[:], dr[i][:])
                    rc.append(t)
            bc = []
            for i in range(4):  # (head band, nh): A0, A1, B0, B1
                t = fin_pool.tile([PART, 512], F32, tag=f"bc{i}", name=f"bc{i}")
                nc.gpsimd.partition_broadcast(t[:], rc[i][:], channels=PART)
                bc.append(t)
            if debug and j == 0:
                nc.sync.dma_start(dbg["dbg_red"][:, 0:512], bc[0][:])
                nc.sync.dma_start(dbg["dbg_red"][:, 512:1024], bc[1][:])
                nc.sync.dma_start(dbg["dbg_red"][:, 1024:1536], dsb[:])
                for i in range(1, 4):
                    nc.sync.dma_start(
                        dbg["dbg_red"][i : i + 1, 1536:2048], dr[i][:]
                    )
            return bc

        def emit_pv_pass(j, nh, e_list, po_t):
            for c in range(NLK):
                for hh in range(2):
                    nc.tensor.matmul(
                        po_t[hh * HD : (hh + 1) * HD, :],
                        vv[c][:, (2 * j + hh) * HD : (2 * j + hh + 1) * HD],
                        e_list[c][:, hh * L + nh * 512 : hh * L + (nh + 1) * 512],
                        start=(c == 0),
                        stop=(c == NLK - 1),
                    )

        e_hold = {}
        bc_hold = {}

        # ============ pair 0 (bootstrap: q/k tiles + v' interleaved) ============
        vb_ctx = ExitStack()
        vb_pool = vb_ctx.enter_context(tc.tile_pool(name="vbp", bufs=2, space="PSUM"))
        # HAM warm-up: ~7us of throwaway matmuls during the input-DMA wait
        # so q0/k0 run at 2.4GHz instead of the cold 1.2GHz default
        scr_sb = singles.tile([PART, 512], BF16)
        nc.vector.memset(scr_sb[:], 1.0)
        warm_ps = st_pool.tile([PART, L], F32, tag="st", name="warm")
        for i in range(16):
            nc.tensor.matmul(
                warm_ps[:, 0:512],
                scr_sb[:, 0:PART],
                scr_sb[:],
                start=(i == 0),
                stop=(i == 15),
            )
        compute_mtile(0, st_pool, tag="st")
        compute_mtile(1, st_pool, tag="st")
        load_mtile(4)
        load_mtile(5)

        # work units woven between pair-0 score chunks (3 per slot)
        units = [("mt", i, p) for i in (2, 3) for p in range(2 * QSPLIT)]
        units += [("v", c, g) for c in range(NLK) for g in range(2)]

        def pop_unit():
            if not units:
                return
            kind, a, b = units.pop(0)
            if kind == "mt":
                compute_mtile(a, vb_pool, tag="vb", part=b)
            else:
                compute_vhalf(a, b, vb_pool)

        e_list0 = []
        for c in range(NLK):
            e_t = e_pool.tile([PART, 2 * L], BF16, tag="e", name="et")
            emit_scores(0, c, e_t)
            e_list0.append(e_t)
            if c >= 1:
                pop_unit()
                pop_unit()
                pop_unit()
            if c == 7:
                load_mtile(6)
                load_mtile(7)
        while units:
            pop_unit()
        e_hold[0] = e_list0
        if debug:
            nc.sync.dma_start(dbg["dbg_q"][:, :], qt[0][:])
            nc.sync.dma_start(dbg["dbg_k"][:, :], kt[0][:])
            nc.sync.dma_start(dbg["dbg_vv"][:, :], vv[0][:])
            nc.sync.dma_start(dbg["dbg_e"][:, :], e_list0[0][:])
        # wo / bout DMAs (needed only in phase 3; wo reuses the wv rings)
        nc.sync.dma_start(bout_sb[:], bout[:, :])
        wo_sb = []
        for k in range(NK):
            t = wv_pool.tile([PART, D], BF16, tag=f"wv{k}", name=f"wo{k}")
            nc.sync.dma_start(t[:], wo_blk[k])
            wo_sb.append(t)
        vb_ctx.close()

        # ============ pairs 1..7 (+ deferred den/PV of pair j-1) ============
        po_ctx = ExitStack()
        po_pool = po_ctx.enter_context(tc.tile_pool(name="pop", bufs=1, space="PSUM"))
        den_ctx = ExitStack()
        den_pool = den_ctx.enter_context(tc.tile_pool(name="denp", bufs=1, space="PSUM"))

        drain_state = {}

        def drain_slice(jp, step, pool=None):
            """One slice of pair jp's deferred PV/normalize work.
            step 0: PV nh0 chunks 0-3 ; 1: PV nh0 4-7 + mul ;
            step 2: PV nh1 chunks 0-3 ; 3: PV nh1 4-7 + mul (frees e tiles)."""
            bc = bc_hold[jp]
            e_list = e_hold[jp]
            nh, half = step // 2, step % 2
            if half == 0:
                pool = pool if pool is not None else po_pool
                drain_state[jp] = pool.tile([PART, 512], F32, tag="po", name="po")
            po_t = drain_state[jp]
            for c in range(half * 4, half * 4 + 4):
                for hh in range(2):
                    nc.tensor.matmul(
                        po_t[hh * HD : (hh + 1) * HD, :],
                        vv[c][:, (2 * jp + hh) * HD : (2 * jp + hh + 1) * HD],
                        e_list[c][:, hh * L + nh * 512 : hh * L + (nh + 1) * 512],
                        start=(c == 0),
                        stop=(c == NLK - 1),
                    )
            if half == 1:
                nsl = slice(nh * 512, (nh + 1) * 512)
                nc.vector.tensor_mul(
                    ot[jp][0:HD, nsl], po_t[0:HD, :], bc[nh][0:HD, :]
                )
                nc.vector.tensor_mul(
                    ot[jp][HD:PART, nsl], po_t[HD:PART, :], bc[2 + nh][HD:PART, :]
                )
                if step == 3:
                    del drain_state[jp]
                    bc_hold.pop(jp)
                    e_hold.pop(jp)
                    if debug and jp == 0:
                        nc.sync.dma_start(dbg["dbg_ot"][:, :], ot[jp][:])

        # pair 0's denominators (deferred: the den bank only frees up now)
        den0 = den_pool.tile([PART, 512], F32, tag="den", name="den")
        for c in range(NLK):
            emit_den(c, e_hold[0][c], den0)
        bc_hold[0] = emit_finish_prep(0, den0)

        for j in range(1, NPAIR):
            den_t = den_pool.tile([PART, 512], F32, tag="den", name="den")
            e_list = []
            for c in range(NLK):
                e_t = e_pool.tile([PART, 2 * L], BF16, tag="e", name="et")
                emit_scores(j, c, e_t)
                e_list.append(e_t)
                if c >= 1:
                    emit_den(c - 1, e_list[c - 1], den_t)
                if c <= 3:
                    drain_slice(j - 1, c)
                if j < NPAIR - 1:
                    # one mtile quarter per slot: qk projections for pair j+1
                    if c < 4:
                        compute_mtile(2 * j + 2, po_pool, tag="po", part=c)
                    else:
                        compute_mtile(2 * j + 3, po_pool, tag="po", part=c - 4)
                    if c == 6:
                        load_mtile(2 * j + 6)
                        load_mtile(2 * j + 7)
            emit_den(NLK - 1, e_list[NLK - 1], den_t)
            e_hold[j] = e_list
            bc_hold[j] = emit_finish_prep(j, den_t)
        st_ctx.close()
        den_ctx.close()

        # ============ phase 3: output projection ============
        # (st/den banks freed; pair-7's drain interleaves with out-proj
        # partials so a bc-gated PV never leaves the PE idle)
        with tc.tile_pool(name="fsb", bufs=2) as f_pool, tc.tile_pool(
            name="po2", bufs=1, space="PSUM"
        ) as po2_pool, tc.tile_pool(
            name="pf", bufs=3, space="PSUM"
        ) as pf_pool:

            def emit_pf(lq, pf_t, ks):
                for k in ks:
                    for nh in range(2):
                        nc.tensor.matmul(
                            pf_t[:, nh * 512 : (nh + 1) * 512],
                            ot[k][:, lq * PART : (lq + 1) * PART],
                            wo_sb[k][:, nh * 512 : (nh + 1) * 512],
                            start=(k == 0),
                            stop=(k == NK - 1),
                        )

            def evac_pf(lq, pf_t):
                # the Scalar queue is ACT-free by phase 3, so alternate the
                # output DMAs across both hardware DGE queues
                fs = f_pool.tile([PART, D], F32, tag="fsb", name="fs")
                for q in range(4):
                    ns = slice(q * 256, (q + 1) * 256)
                    nc.vector.tensor_add(fs[:, ns], pf_t[:, ns], bout_sb[:, ns])
                    eng = nc.sync if q % 2 == 0 else nc.scalar
                    eng.dma_start(Y[lq * PART : (lq + 1) * PART, ns], fs[:, ns])

            drain_slice(NPAIR - 1, 0)
            pf0 = pf_pool.tile([PART, D], F32, tag="pf", name="pf")
            emit_pf(0, pf0, range(NK - 1))
            drain_slice(NPAIR - 1, 1)
            pf1 = pf_pool.tile([PART, D], F32, tag="pf", name="pf")
            emit_pf(1, pf1, range(NK - 1))
            drain_slice(NPAIR - 1, 2, pool=po2_pool)
            pf2 = pf_pool.tile([PART, D], F32, tag="pf", name="pf")
            emit_pf(2, pf2, range(NK - 1))
            drain_slice(NPAIR - 1, 3)
            emit_pf(0, pf0, [NK - 1])
            evac_pf(0, pf0)
            emit_pf(1, pf1, [NK - 1])
            evac_pf(1, pf1)
            emit_pf(2, pf2, [NK - 1])
            evac_pf(2, pf2)
            for lq in range(3, NLQ):
                pf_t = pf_pool.tile([PART, D], F32, tag="pf", name="pf")
                emit_pf(lq, pf_t, range(NK))
                evac_pf(lq, pf_t)
        po_ctx.close()

    nc.compile()
    return nc


_NC_CACHE = None


def _get_nc():
    global _NC_CACHE
    if _NC_CACHE is None:
        _NC_CACHE = build_nc()
    return _NC_CACHE


def make_in_maps(x, attn_mask, w_qkv, b_qkv, w_out, b_out):
    """Host-side sharding + layout prep -> per-core input maps."""
    import ml_dtypes

    bf16 = ml_dtypes.bfloat16
    x = np.asarray(x, dtype=np.float32)
    attn_mask = np.asarray(attn_mask)
    w_qkv = np.asarray(w_qkv, dtype=np.float32)
    b_qkv = np.asarray(b_qkv, dtype=np.float32)
    w_out = np.asarray(w_out, dtype=np.float32)
    b_out = np.asarray(b_out, dtype=np.float32)

    wqk = w_qkv[:, : 2 * D]  # (D, 2D)
    wqk_blk = np.ascontiguousarray(
        wqk.reshape(NK, PART, NM, PART).transpose(2, 1, 0, 3).astype(bf16)
    )
    bqk_h = np.ascontiguousarray(b_qkv[: 2 * D].reshape(NM, PART).T).astype(np.float32)

    wv = w_qkv[:, 2 * D :]  # (D, D)
    wv_blk_h = np.ascontiguousarray(wv.reshape(NK, PART, D).astype(bf16))
    vbias_h = np.ascontiguousarray(
        np.broadcast_to(b_qkv[2 * D :], (PART, D))
    ).astype(np.float32)

    maskbias = np.where(attn_mask.astype(bool), 0.0, -10000.0).astype(np.float32)

    wo_blk_h = np.ascontiguousarray(w_out.reshape(NK, PART, D).astype(bf16))
    bout_h = np.ascontiguousarray(np.broadcast_to(b_out, (PART, D))).astype(np.float32)

    in_maps = []
    for b in range(B):
        in_maps.append(
            {
                "xT": np.ascontiguousarray(x[b].T.astype(bf16)),
                "wqk_blk": wqk_blk,
                "bqk": bqk_h,
                "wv_blk": wv_blk_h,
                "vbias": vbias_h,
                "maskb": np.ascontiguousarray(maskbias[b].reshape(NLK, PART).T),
                "wo_blk": wo_blk_h,
                "bout": bout_h,
            }
        )
    return in_maps


def kernel(x, attn_mask, w_qkv, b_qkv, w_out, b_out):
    in_maps = make_in_maps(x, attn_mask, w_qkv, b_qkv, w_out, b_out)
    nc = _get_nc()
    res = run_bass_kernel_spmd(nc, in_maps, core_ids=list(range(N_CORES)))
    return np.stack([res.results[b]["Y"] for b in range(B)], axis=0)


if __name__ == "__main__":
    rng = np.random.default_rng(0)
    inputs = {
        "x": rng.standard_normal((B, L, D), dtype=np.float32),
        "attn_mask": np.ones((B, L), dtype=bool),
        "w_qkv": ((rng.random((D, 3 * D), dtype=np.float32) - 0.5) / 16.0),
        "b_qkv": np.zeros((3 * D,), dtype=np.float32),
        "w_out": ((rng.random((D, D), dtype=np.float32) - 0.5) / 16.0),
        "b_out": np.zeros((D,), dtype=np.float32),
    }
    y = kernel(**inputs)
    print(y.shape, y.dtype)


# revision 66
# speedup vs baseline: 1.0932x; 1.0932x over previous
"""Multi-head self-attention Trainium2 kernel (Bass/Tile), batch-parallel
over 8 NeuronCores: one batch element per core, full weights everywhere,
no collectives.

Problem (hardcoded): B=8, L=1024, D=1024, H=16, hd=64, f32 in/out.
  qkv = x @ w_qkv + b_qkv ; per-head scores = q k^T / 8 ; mask ; softmax ;
  out = (P v) heads-merged @ w_out + b_out.

Design notes (v4 — all-bf16, ScalarE-bound softmax loop):
  - ScalarE exp over the 16 x 1024 x 1024 score matrix is the hard floor
    (~(N+352)/1.2 ns per ACT ~= 147us total). The head-pair loop streams
    exp ACTs back-to-back; all matmul work hides under it on TensorE.
    (fp8/DoubleRow was tried: ~2x PE-cheaper but any fp8 stage alone
    costs 2-5e-2 relative error vs the 2e-2 gate — measured proj8=4.8e-2,
    e8=2.4e-2, ot8=3e-2. So everything stays bf16.)
  - Scores: per head K=64; the head pair occupies partition ranges
    0:64 / 64:128 of qt/kt, so the pair's score matmuls row-tile and run
    concurrently in the PE array.
  - PV col-tiles the pair (head A -> PSUM partitions 0:64 via tile col 0,
    head B -> 64:128 via col 64) in two 512-query passes through a single
    1-bank [128, 512] ring; 2x PE concurrency, no ones-column.
  - Softmax denominators come from ones-stationary M=1 matmuls, 4-way
    col-tiled (positions 0/32/64/96 hold the (head, nh) groups), one
    PSUM bank, accumulated chunk-by-chunk alongside the scores. The
    finish chain (PSUM->SBUF copy, row hops to partition 0, fast
    reciprocal, partition_broadcast bands) runs on DVE/Sync/GpSimd
    entirely off the ACT path. partition_broadcast only reads/writes
    from physical partition 0, hence the row-hop DMAs.
  - PV + normalize for pair j are deferred into pair j+1's chunk slots
    (e tiles held one extra pair) and emitted in 4 slices so a pending
    PV at the PE queue head never delays the score matmuls feeding the
    next exp ACT.
  - QKV m-tiles for pair j+1 are computed during pair j as 512-token
    halves through the same po ring (1 bank), freeing 2 banks vs a
    dedicated pool; that buys st bufs=3, whose 3-deep score ladder
    absorbs PE bursts up to ~2.3us without stalling ScalarE.
  - PSUM budget (8 banks): scores st 3x2 + po ring 1 + den 1 = 8.
    Pair 0 bootstraps v' (16 chunk-halves) + q/k tiles through a 2-bank
    ring woven 3-per-slot between its own score chunks; its denominator
    matmuls are deferred to the pair-1 boundary where the den bank opens.
  - All evacuations run on DVE, keeping ScalarE exp-only. wo tiles reuse
    the wv pool rings (wv is dead after the v' chunks). The tail
    interleaves pair-7's drain with out-proj partials (k=0..6) so the
    reciprocal-chain wait never idles the PE.
"""

import sys

import numpy as np

try:
    import concourse.bass as bass  # noqa: F401
except Exception:  # pragma: no cover - defensive path setup
    for p in ("/opt/trn_rl_repo", "/opt/pypackages"):
        if p not in sys.path:
            sys.path.insert(0, p)
    import concourse.bass as bass  # noqa: F401

from contextlib import ExitStack

import concourse.tile as tile
from concourse import bacc, bass_isa, mybir
from concourse.bass_utils import run_bass_kernel_spmd

F32 = mybir.dt.float32
BF16 = mybir.dt.bfloat16

B, L, D = 8, 1024, 1024
H, HD = 16, 64
N_CORES = 8
PART = 128
NK = D // PART  # 8 x-dim contraction chunks
NM = 2 * D // PART  # 16 qk output tiles (q: 0-7, k: 8-15)
NLQ = L // PART  # 8 query tiles
NLK = L // PART  # 8 key tiles
NPAIR = H // 2  # 8 head pairs
EXP = mybir.ActivationFunctionType.Exp
RADD = bass_isa.ReduceOp.add


def build_nc(debug=False):
    nc = bacc.Bacc("TRN2", target_bir_lowering=False, debug=False)

    xT = nc.dram_tensor("xT", (D, L), BF16, kind="ExternalInput").ap()
    # wqk_blk[m, p, k, c] = w_qkv[k*128 + p, m*128 + c], m < 16 (q then k)
    wqk_blk = nc.dram_tensor(
        "wqk_blk", (NM, PART, NK, PART), BF16, kind="ExternalInput"
    ).ap()
    # bqk[p, m] = b_qkv[m*128 + p]
    bqk = nc.dram_tensor("bqk", (PART, NM), F32, kind="ExternalInput").ap()
    # wv_blk[k, p, n] = w_v[k*128 + p, n]
    wv_blk = nc.dram_tensor("wv_blk", (NK, PART, D), BF16, kind="ExternalInput").ap()
    # vbias[p, n] = b_v[n] (broadcast over p)
    vbias = nc.dram_tensor("vbias", (PART, D), F32, kind="ExternalInput").ap()
    # maskb[p, c] = 0 / -10000 for key c*128+p
    maskb = nc.dram_tensor("maskb", (PART, NLK), F32, kind="ExternalInput").ap()
    # wo_blk[k, p, n] = w_out[k*128 + p, n]
    wo_blk = nc.dram_tensor("wo_blk", (NK, PART, D), BF16, kind="ExternalInput").ap()
    bout = nc.dram_tensor("bout", (PART, D), F32, kind="ExternalInput").ap()
    Y = nc.dram_tensor("Y", (L, D), F32, kind="ExternalOutput").ap()
    dbg = {}
    if debug:
        for nm, shp, dt in [
            ("dbg_q", (PART, L), BF16), ("dbg_k", (PART, L), BF16),
            ("dbg_vv", (PART, D), BF16), ("dbg_e", (PART, 2 * L), BF16),
            ("dbg_esum", (PART, 2 * L), F32), ("dbg_red", (PART, 2 * L), F32),
            ("dbg_rcp", (PART, 2 * L), F32), ("dbg_ot", (PART, L), BF16),
        ]:
            dbg[nm] = nc.dram_tensor(nm, shp, dt, kind="ExternalOutput").ap()

    with tile.TileContext(nc) as tc, ExitStack() as ctx:
        singles = ctx.enter_context(tc.tile_pool(name="singles", bufs=1))
        mb_sb = singles.tile([PART, NLK], F32)
        nc.sync.dma_start(mb_sb[:], maskb[:, :])
        bqk_sb = singles.tile([PART, NM], F32)
        nc.sync.dma_start(bqk_sb[:], bqk[:, :])
        vbias_sb = singles.tile([PART, D], F32)
        bout_sb = singles.tile([PART, D], F32)

        # ---- persistent tiles ----
        xt_pool = ctx.enter_context(tc.tile_pool(name="xt", bufs=1))
        xt = [xt_pool.tile([PART, L], BF16, tag=f"xt{k}", name=f"xt{k}") for k in range(NK)]
        wv_pool = ctx.enter_context(tc.tile_pool(name="wv", bufs=1))
        wv_sb = [wv_pool.tile([PART, D], BF16, tag=f"wv{k}", name=f"wv{k}") for k in range(NK)]
        qk_pool = ctx.enter_context(tc.tile_pool(name="qk", bufs=1))
        qt = [qk_pool.tile([PART, L], BF16, tag=f"q{j}", name=f"q{j}") for j in range(NPAIR)]
        kt = [qk_pool.tile([PART, L], BF16, tag=f"k{j}", name=f"k{j}") for j in range(NPAIR)]
        vv_pool = ctx.enter_context(tc.tile_pool(name="vv", bufs=1))
        vv = [vv_pool.tile([PART, D], BF16, tag=f"vv{c}", name=f"vv{c}") for c in range(NLK)]
        ot_pool = ctx.enter_context(tc.tile_pool(name="ot", bufs=1))
        ot = [ot_pool.tile([PART, L], BF16, tag=f"ot{j}", name=f"ot{j}") for j in range(NPAIR)]

        # qk weight stream
        wqk_pool = ctx.enter_context(tc.tile_pool(name="wqkp", bufs=4))
        mt_seq = [j + NPAIR * s for j in range(NPAIR) for s in range(2)]  # q0,k0,q1,k1,...
        mt_loaded = {}

        def load_mtile(i):
            if i >= NM:
                return
            m = mt_seq[i]
            wt = wqk_pool.tile([PART, NK * PART], BF16, tag="wqk", name="wt")
            nc.sync.dma_start(wt[:], wqk_blk[m].rearrange("p k c -> p (k c)"))
            mt_loaded[i] = wt

        def compute_mtile(i, psum_pool, tag="mt", part=None):
            """part=None: whole [128,L] tile (2 banks).
            part=0/1: token-halves through a 1-bank [128,512] ring."""
            if i >= NM:
                return None
            m = mt_seq[i]
            dst = qt[m] if m < NPAIR else kt[m - NPAIR]
            if part is None:
                wt = mt_loaded.pop(i)
                ph = psum_pool.tile([PART, L], F32, tag=tag, name="ph")
                for k in range(NK):
                    for nh in range(2):
                        nc.tensor.matmul(
                            ph[:, nh * 512 : (nh + 1) * 512],
                            wt[:, k * PART : (k + 1) * PART],
                            xt[k][:, nh * 512 : (nh + 1) * 512],
                            start=(k == 0),
                            stop=(k == NK - 1),
                        )
                nc.vector.tensor_scalar_add(dst[:], ph[:], bqk_sb[:, m : m + 1])
                return
            wt = mt_loaded[i]
            nsl = slice(part * 512, (part + 1) * 512)
            ph = psum_pool.tile([PART, 512], F32, tag=tag, name="ph")
            for k in range(NK):
                nc.tensor.matmul(
                    ph[:],
                    wt[:, k * PART : (k + 1) * PART],
                    xt[k][:, nsl],
                    start=(k == 0),
                    stop=(k == NK - 1),
                )
            nc.vector.tensor_scalar_add(dst[:, nsl], ph[:], bqk_sb[:, m : m + 1])
            if part == 1:
                del mt_loaded[i]

        def compute_vchunk(c, psum_pool, tag="vb"):
            pv = psum_pool.tile([PART, D], F32, tag=tag, name="pv")
            for k in range(NK):
                for g in range(2):
                    nc.tensor.matmul(
                        pv[:, g * 512 : (g + 1) * 512],
                        xt[k][:, c * PART : (c + 1) * PART],
                        wv_sb[k][:, g * 512 : (g + 1) * 512],
                        start=(k == 0),
                        stop=(k == NK - 1),
                    )
            nc.vector.tensor_add(vv[c][:], pv[:], vbias_sb[:])

        def compute_vhalf(c, g, psum_pool):
            """Half of v' chunk c (512 of 1024 v-dims) -> vv[c]."""
            gsl = slice(g * 512, (g + 1) * 512)
            pv = psum_pool.tile([PART, 512], F32, tag="vb", name="pv")
            for k in range(NK):
                nc.tensor.matmul(
                    pv[:],
                    xt[k][:, c * PART : (c + 1) * PART],
                    wv_sb[k][:, gsl],
                    start=(k == 0),
                    stop=(k == NK - 1),
                )
            nc.vector.tensor_add(vv[c][:, gsl], pv[:], vbias_sb[:, gsl])

        # ============ input DMAs (sync queue, consumption order) ============
        load_mtile(0)
        load_mtile(1)
        for k in range(NK):
            nc.sync.dma_start(xt[k][:], xT[k * PART : (k + 1) * PART, :])
        load_mtile(2)
        load_mtile(3)
        for k in range(NK):
            nc.sync.dma_start(wv_sb[k][:], wv_blk[k])
        nc.sync.dma_start(vbias_sb[:], vbias[:, :])

        # ============ SBUF pools for the loop ============
        st_ctx = ExitStack()
        st_pool = st_ctx.enter_context(
            tc.tile_pool(name="stp", bufs=3, space="PSUM", side="right")
        )
        e_pool = ctx.enter_context(tc.tile_pool(name="epool", bufs=11))
        fin_pool = ctx.enter_context(tc.tile_pool(name="fin", bufs=1))
        ones_sb = singles.tile([PART, 1], BF16)
        nc.vector.memset(ones_sb[:], 1.0)

        def emit_scores(j, c, e_t):
            """Score matmuls + exp ACTs for chunk c of head pair j."""
            csl = slice(c * PART, (c + 1) * PART)
            for hh in range(2):  # head A (rows 0:64) / head B (64:128)
                ro = hh * HD
                st = st_pool.tile([PART, L], F32, tag="st", name="st")
                for nh in range(2):
                    nsl = slice(nh * 512, (nh + 1) * 512)
                    nc.tensor.matmul(
                        st[:, nsl],
                        kt[j][ro : ro + HD, csl],
                        qt[j][ro : ro + HD, nsl],
                        start=True,
                        stop=True,
                    )
                nc.scalar.activation(
                    e_t[:, hh * L : (hh + 1) * L],
                    st[:],
                    EXP,
                    bias=mb_sb[:, c : c + 1],
                    scale=1.0 / 8.0,
                )

        def emit_den(c, e_t, den_t):
            """Accumulate softmax denominators for chunk c into the den PSUM
            tile: ones-stationary M=1 matmuls, 4-way col-tiled so the four
            (head, nh) groups run concurrently at array columns 0/32/64/96."""
            for hh in range(2):
                for nh in range(2):
                    pos = 32 * (2 * hh + nh)
                    nc.tensor.matmul(
                        den_t[pos : pos + 1, :],
                        ones_sb[:],
                        e_t[:, hh * L + nh * 512 : hh * L + (nh + 1) * 512],
                        start=(c == 0),
                        stop=(c == NLK - 1),
                        tile_position=(0, pos),
                    )

        def emit_finish_prep(j, den_t):
            """den PSUM rows 0/32/64/96 -> partition-0 tiles -> recip ->
            broadcast bands. Frees the den bank via small DMAs."""
            dsb = fin_pool.tile([PART, 512], F32, tag="dsb", name="dsb")
            nc.vector.tensor_copy(dsb[:], den_t[:])
            dr, rc = [dsb[0:1, :]], []
            for i in range(1, 4):
                t = fin_pool.tile([1, 512], F32, tag=f"dr{i}", name=f"dr{i}")
                nc.sync.dma_start(t[:], dsb[32 * i : 32 * i + 1, :])
                dr.append(t)
            with nc.allow_low_precision(reason="softmax denom reciprocal"):
                for i in range(4):
                    t = fin_pool.tile([1, 512], F32, tag=f"rc{i}", name=f"rc{i}")
                    nc.vector.reciprocal_approx_fast(t[:], dr[i][:])
                    rc.append(t)
            bc = []
            for i in range(4):  # (head band, nh): A0, A1, B0, B1
                t = fin_pool.tile([PART, 512], F32, tag=f"bc{i}", name=f"bc{i}")
                nc.gpsimd.partition_broadcast(t[:], rc[i][:], channels=PART)
                bc.append(t)
            if debug and j == 0:
                nc.sync.dma_start(dbg["dbg_red"][:, 0:512], bc[0][:])
                nc.sync.dma_start(dbg["dbg_red"][:, 512:1024], bc[1][:])
                nc.sync.dma_start(dbg["dbg_red"][:, 1024:1536], dsb[:])
                for i in range(1, 4):
                    nc.sync.dma_start(
                        dbg["dbg_red"][i : i + 1, 1536:2048], dr[i][:]
                    )
            return bc

        def emit_pv_pass(j, nh, e_list, po_t):
            for c in range(NLK):
                for hh in range(2):
                    nc.tensor.matmul(
                        po_t[hh * HD : (hh + 1) * HD, :],
                        vv[c][:, (2 * j + hh) * HD : (2 * j + hh + 1) * HD],
                        e_list[c][:, hh * L + nh * 512 : hh * L + (nh + 1) * 512],
                        start=(c == 0),
                        stop=(c == NLK - 1),
                    )

        e_hold = {}
        bc_hold = {}

        # ============ pair 0 (bootstrap: q/k tiles + v' interleaved) ============
        vb_ctx = ExitStack()
        vb_pool = vb_ctx.enter_context(tc.tile_pool(name="vbp", bufs=2, space="PSUM"))
        # HAM warm-up: ~7us of throwaway matmuls during the input-DMA wait
        # so q0/k0 run at 2.4GHz instead of the cold 1.2GHz default
        scr_sb = singles.tile([PART, 512], BF16)
        nc.vector.memset(scr_sb[:], 1.0)
        warm_ps = st_pool.tile([PART, L], F32, tag="st", name="warm")
        for i in range(16):
            nc.tensor.matmul(
                warm_ps[:, 0:512],
                scr_sb[:, 0:PART],
                scr_sb[:],
                start=(i == 0),
                stop=(i == 15),
            )
        compute_mtile(0, st_pool, tag="st")
        compute_mtile(1, st_pool, tag="st")
        load_mtile(4)
        load_mtile(5)

        # work units woven between pair-0 score chunks (3 per slot)
        units = [("mt", 2, 0), ("mt", 2, 1), ("mt", 3, 0), ("mt", 3, 1)]
        units += [("v", c, g) for c in range(NLK) for g in range(2)]

        def pop_unit():
            if not units:
                return
            kind, a, b = units.pop(0)
            if kind == "mt":
                compute_mtile(a, vb_pool, tag="vb", part=b)
            else:
                compute_vhalf(a, b, vb_pool)

        e_list0 = []
        for c in range(NLK):
            e_t = e_pool.tile([PART, 2 * L], BF16, tag="e", name="et")
            emit_scores(0, c, e_t)
            e_list0.append(e_t)
            if c >= 1:
                pop_unit()
                pop_unit()
                pop_unit()
            if c == 7:
                load_mtile(6)
                load_mtile(7)
        e_hold[0] = e_list0
        if debug:
            nc.sync.dma_start(dbg["dbg_q"][:, :], qt[0][:])
            nc.sync.dma_start(dbg["dbg_k"][:, :], kt[0][:])
            nc.sync.dma_start(dbg["dbg_vv"][:, :], vv[0][:])
            nc.sync.dma_start(dbg["dbg_e"][:, :], e_list0[0][:])
        # wo / bout DMAs (needed only in phase 3; wo reuses the wv rings)
        nc.sync.dma_start(bout_sb[:], bout[:, :])
        wo_sb = []
        for k in range(NK):
            t = wv_pool.tile([PART, D], BF16, tag=f"wv{k}", name=f"wo{k}")
            nc.sync.dma_start(t[:], wo_blk[k])
            wo_sb.append(t)
        vb_ctx.close()

        # ============ pairs 1..7 (+ deferred den/PV of pair j-1) ============
        po_ctx = ExitStack()
        po_pool = po_ctx.enter_context(tc.tile_pool(name="pop", bufs=1, space="PSUM"))
        den_ctx = ExitStack()
        den_pool = den_ctx.enter_context(tc.tile_pool(name="denp", bufs=1, space="PSUM"))

        drain_state = {}

        def drain_slice(jp, step, pool=None):
            """One slice of pair jp's deferred PV/normalize work.
            step 0: PV nh0 chunks 0-3 ; 1: PV nh0 4-7 + mul ;
            step 2: PV nh1 chunks 0-3 ; 3: PV nh1 4-7 + mul (frees e tiles)."""
            bc = bc_hold[jp]
            e_list = e_hold[jp]
            nh, half = step // 2, step % 2
            if half == 0:
                pool = pool if pool is not None else po_pool
                drain_state[jp] = pool.tile([PART, 512], F32, tag="po", name="po")
            po_t = drain_state[jp]
            for c in range(half * 4, half * 4 + 4):
                for hh in range(2):
                    nc.tensor.matmul(
                        po_t[hh * HD : (hh + 1) * HD, :],
                        vv[c][:, (2 * jp + hh) * HD : (2 * jp + hh + 1) * HD],
                        e_list[c][:, hh * L + nh * 512 : hh * L + (nh + 1) * 512],
                        start=(c == 0),
                        stop=(c == NLK - 1),
                    )
            if half == 1:
                nsl = slice(nh * 512, (nh + 1) * 512)
                nc.vector.tensor_mul(
                    ot[jp][0:HD, nsl], po_t[0:HD, :], bc[nh][0:HD, :]
                )
                nc.vector.tensor_mul(
                    ot[jp][HD:PART, nsl], po_t[HD:PART, :], bc[2 + nh][HD:PART, :]
                )
                if step == 3:
                    del drain_state[jp]
                    bc_hold.pop(jp)
                    e_hold.pop(jp)
                    if debug and jp == 0:
                        nc.sync.dma_start(dbg["dbg_ot"][:, :], ot[jp][:])

        # pair 0's denominators (deferred: the den bank only frees up now)
        den0 = den_pool.tile([PART, 512], F32, tag="den", name="den")
        for c in range(NLK):
            emit_den(c, e_hold[0][c], den0)
        bc_hold[0] = emit_finish_prep(0, den0)

        for j in range(1, NPAIR):
            den_t = den_pool.tile([PART, 512], F32, tag="den", name="den")
            e_list = []
            for c in range(NLK):
                e_t = e_pool.tile([PART, 2 * L], BF16, tag="e", name="et")
                emit_scores(j, c, e_t)
                e_list.append(e_t)
                if c >= 1:
                    emit_den(c - 1, e_list[c - 1], den_t)
                if c <= 3:
                    drain_slice(j - 1, c)
                if c >= 4 and j < NPAIR - 1:
                    # slots 4-7: next pair's qk projections, token-halves
                    # through the po ring (1 bank each)
                    if c in (4, 5):
                        compute_mtile(2 * j + 2, po_pool, tag="po", part=c - 4)
                    else:
                        compute_mtile(2 * j + 3, po_pool, tag="po", part=c - 6)
                    if c == 6:
                        load_mtile(2 * j + 6)
                        load_mtile(2 * j + 7)
            emit_den(NLK - 1, e_list[NLK - 1], den_t)
            e_hold[j] = e_list
            bc_hold[j] = emit_finish_prep(j, den_t)
        st_ctx.close()
        den_ctx.close()

        # ============ phase 3: output projection ============
        # (st/den banks freed; pair-7's drain interleaves with out-proj
        # partials so a bc-gated PV never leaves the PE idle)
        with tc.tile_pool(name="fsb", bufs=2) as f_pool, tc.tile_pool(
            name="po2", bufs=1, space="PSUM"
        ) as po2_pool, tc.tile_pool(
            name="pf", bufs=3, space="PSUM"
        ) as pf_pool:

            def emit_pf(lq, pf_t, ks):
                for k in ks:
                    for nh in range(2):
                        nc.tensor.matmul(
                            pf_t[:, nh * 512 : (nh + 1) * 512],
                            ot[k][:, lq * PART : (lq + 1) * PART],
                            wo_sb[k][:, nh * 512 : (nh + 1) * 512],
                            start=(k == 0),
                            stop=(k == NK - 1),
                        )

            def evac_pf(lq, pf_t):
                # the Scalar queue is ACT-free by phase 3, so alternate the
                # output DMAs across both hardware DGE queues
                fs = f_pool.tile([PART, D], F32, tag="fsb", name="fs")
                for q in range(4):
                    ns = slice(q * 256, (q + 1) * 256)
                    nc.vector.tensor_add(fs[:, ns], pf_t[:, ns], bout_sb[:, ns])
                    eng = nc.sync if q % 2 == 0 else nc.scalar
                    eng.dma_start(Y[lq * PART : (lq + 1) * PART, ns], fs[:, ns])

            drain_slice(NPAIR - 1, 0)
            pf0 = pf_pool.tile([PART, D], F32, tag="pf", name="pf")
            emit_pf(0, pf0, range(NK - 1))
            drain_slice(NPAIR - 1, 1)
            pf1 = pf_pool.tile([PART, D], F32, tag="pf", name="pf")
            emit_pf(1, pf1, range(NK - 1))
            drain_slice(NPAIR - 1, 2, pool=po2_pool)
            pf2 = pf_pool.tile([PART, D], F32, tag="pf", name="pf")
            emit_pf(2, pf2, range(NK - 1))
            drain_slice(NPAIR - 1, 3)
            emit_pf(0, pf0, [NK - 1])
            evac_pf(0, pf0)
            emit_pf(1, pf1, [NK - 1])
            evac_pf(1, pf1)
            emit_pf(2, pf2, [NK - 1])
            evac_pf(2, pf2)
            for lq in range(3, NLQ):
                pf_t = pf_pool.tile([PART, D], F32, tag="pf", name="pf")
                emit_pf(lq, pf_t, range(NK))
                evac_pf(lq, pf_t)
        po_ctx.close()

    nc.compile()
    return nc


_NC_CACHE = None


def _get_nc():
    global _NC_CACHE
    if _NC_CACHE is None:
        _NC_CACHE = build_nc()
    return _NC_CACHE


def make_in_maps(x, attn_mask, w_qkv, b_qkv, w_out, b_out):
    """Host-side sharding + layout prep -> per-core input maps."""
    import ml_dtypes

    bf16 = ml_dtypes.bfloat16
    x = np.asarray(x, dtype=np.float32)
    attn_mask = np.asarray(attn_mask)
    w_qkv = np.asarray(w_qkv, dtype=np.float32)
    b_qkv = np.asarray(b_qkv, dtype=np.float32)
    w_out = np.asarray(w_out, dtype=np.float32)
    b_out = np.asarray(b_out, dtype=np.float32)

    wqk = w_qkv[:, : 2 * D]  # (D, 2D)
    wqk_blk = np.ascontiguousarray(
        wqk.reshape(NK, PART, NM, PART).transpose(2, 1, 0, 3).astype(bf16)
    )
    bqk_h = np.ascontiguousarray(b_qkv[: 2 * D].reshape(NM, PART).T).astype(np.float32)

    wv = w_qkv[:, 2 * D :]  # (D, D)
    wv_blk_h = np.ascontiguousarray(wv.reshape(NK, PART, D).astype(bf16))
    vbias_h = np.ascontiguousarray(
        np.broadcast_to(b_qkv[2 * D :], (PART, D))
    ).astype(np.float32)

    maskbias = np.where(attn_mask.astype(bool), 0.0, -10000.0).astype(np.float32)

    wo_blk_h = np.ascontiguousarray(w_out.reshape(NK, PART, D).astype(bf16))
    bout_h = np.ascontiguousarray(np.broadcast_to(b_out, (PART, D))).astype(np.float32)

    in_maps = []
    for b in range(B):
        in_maps.append(
            {
                "xT": np.ascontiguousarray(x[b].T.astype(bf16)),
                "wqk_blk": wqk_blk,
                "bqk": bqk_h,
                "wv_blk": wv_blk_h,
                "vbias": vbias_h,
                "maskb": np.ascontiguousarray(maskbias[b].reshape(NLK, PART).T),
                "wo_blk": wo_blk_h,
                "bout": bout_h,
            }
        )
    return in_maps


def kernel(x, attn_mask, w_qkv, b_qkv, w_out, b_out):
    in_maps = make_in_maps(x, attn_mask, w_qkv, b_qkv, w_out, b_out)
    nc = _get_nc()
    res = run_bass_kernel_spmd(nc, in_maps, core_ids=list(range(N_CORES)))
    return np.stack([res.results[b]["Y"] for b in range(B)], axis=0)


if __name__ == "__main__":
    rng = np.random.default_rng(0)
    inputs = {
        "x": rng.standard_normal((B, L, D), dtype=np.float32),
        "attn_mask": np.ones((B, L), dtype=bool),
        "w_qkv": ((rng.random((D, 3 * D), dtype=np.float32) - 0.5) / 16.0),
        "b_qkv": np.zeros((3 * D,), dtype=np.float32),
        "w_out": ((rng.random((D, D), dtype=np.float32) - 0.5) / 16.0),
        "b_out": np.zeros((D,), dtype=np.float32),
    }
    y = kernel(**inputs)
    print(y.shape, y.dtype)
